# revision 25
# baseline (speedup 1.0000x reference)
import numpy as np
import concourse.bass as bass
import concourse.tile as tile
from concourse import mybir
from concourse.bass_utils import run_bass_kernel_spmd
from concourse.masks import make_identity

P = 128
S = 2048
D = 512
U = 1024
NS = S // P      # 16 s-tiles
ND = D // P      # 4 d-blocks
NC_ = S // D     # 4 s-chunks
NEG = -60000.0
EPS = 1e-6


def _patched_drain_and_barrier(self, tick_clock, wait_clock):
    nc = self.nc
    probe = nc.sync.nop(nofuse=True, hint="drain_waits_probe")
    wait_clock.add_sem_waits(probe.ins, tile.ScopedClock({None: tick_clock.global_clock}))
    si = probe.ins.sync_info
    waits = list(si.on_wait) if si is not None else []
    assert self.sems is not None
    handles = {h.name: h for h in self.sems.allocated().values()}
    if len(waits) > 1:
        import bass_rust
        probe.ins.sync_info = bass_rust.SyncInfo(on_wait=waits[:1], on_update=[])
        for w in waits[1:]:
            h = handles.get(w.ant_name)
            assert h is not None, (w.ant_name, list(handles))
            nc.sync.wait_ge(h, w.wait_value)
    nc.sync.drain()
    nc.all_engine_barrier()
    popped = nc._tile_sem_poison_stack.pop()
    assert popped is self._sem_poison
    nc.clear_and_free_semaphores(list(self.sems.allocated().values()))
    nc.all_engine_barrier()


tile.TileContext._drain_and_barrier = _patched_drain_and_barrier

# The walrus backend in this toolchain rejects instructions carrying more
# than one semaphore wait ("Too many sync wait commands"). Split excess
# waits onto single-wait NoOp carriers on the same engine, which execute
# in order ahead of the real instruction.
_MAXW = 1
_orig_lower_ordered = tile.TileContext._lower_ordered_insts


def _patched_lower_ordered(self, ordered):
    nc = self.nc
    for insts in ordered.values():
        out = []
        for inst in insts:
            si = getattr(inst, "sync_info", None)
            eng = getattr(inst, "engine", None)
            if (si is not None and si.on_wait and len(si.on_wait) > _MAXW
                    and eng is not None
                    and not type(inst).__name__.startswith("BassTile")):
                waits = list(si.on_wait)
                for w in waits[:-_MAXW]:
                    out.append(mybir.InstNoOp(
                        name=nc.get_next_instruction_name(),
                        engine=eng,
                        ins=[],
                        outs=[],
                        bass_nofuse=True,
                        sync_info=mybir.SyncInfo(on_wait=[w], on_update=[]),
                    ))
                inst.sync_info = mybir.SyncInfo(
                    on_wait=waits[-_MAXW:], on_update=list(si.on_update))
            out.append(inst)
        insts[:] = out
    return _orig_lower_ordered(self, ordered)


tile.TileContext._lower_ordered_insts = _patched_lower_ordered

f32 = mybir.dt.float32
f16 = mybir.dt.float16
bf16 = mybir.dt.bfloat16


def _build():
    nc = bass.Bass()
    # Per-core inputs (1 batch element, 2 heads):
    #   x   [S, D]    activations
    #   ub  [P, 2*ND] per-head score key-side bias (beta @ Wq @ (g*Wk)^T),
    #                 column h*ND+j holds entries d = j*128 + p
    #   a   [2D, D]   A_h = (g*Wq_h)(g*Wk_h)^T stacked over the 2 heads, f16
    #   n   [2D, D]   N_h = (g*Wv_h) Wout_h stacked, f16
    # scoresT = z (z A + u)^T computed transposed so exp writes probsT
    # directly; outputs are UNNORMALIZED probs@vm per head plus the softmax
    # row-sums z-vector; the host divides and sums heads.
    x_ext = nc.declare_dram_parameter("x", [S, D], f32, isOutput=False)
    ub_ext = nc.declare_dram_parameter("ub", [P, 2 * ND], f32, isOutput=False)
    a_ext = nc.declare_dram_parameter("a", [2 * D, D], f16, isOutput=False)
    n_ext = nc.declare_dram_parameter("n", [2 * D, D], f16, isOutput=False)
    # each row: [d0..d255, rowsum, d256..d511, rowsum] (PV folds the softmax
    # denominator in via a ones-column in vm)
    out_ext = nc.declare_dram_parameter("out", [2 * S, 2 * (D // 2 + 1)], bf16, isOutput=True)

    with tile.TileContext(nc) as tc:
        with tc.tile_pool(name="const", bufs=1) as cp, \
             tc.tile_pool(name="znt", bufs=1) as xp, \
             tc.tile_pool(name="wp", bufs=1) as wp, \
             tc.tile_pool(name="qkv", bufs=1) as qp, \
             tc.tile_pool(name="pb", bufs=2) as pbp, \
             tc.tile_pool(name="ln", bufs=2) as lp, \
             tc.tile_pool(name="xd", bufs=16) as xdp, \
             tc.tile_pool(name="outp", bufs=3) as up, \
             tc.tile_pool(name="mm", bufs=2, space="PSUM") as mmp, \
             tc.tile_pool(name="sc", bufs=3, space="PSUM") as scp, \
             tc.tile_pool(name="pva", bufs=1, space="PSUM") as pvap, \
             tc.tile_pool(name="pvb", bufs=1, space="PSUM") as pvbp, \
             tc.tile_pool(name="trl", bufs=1, space="PSUM") as trlp:

            dmaq = [nc.sync, nc.scalar, nc.gpsimd]

            # ---- stage all DMAs up front: x tiles first, weights interleaved ----
            xts = [xdp.tile([P, D], f32, tag="x", name=f"xt{i}") for i in range(NS)]

            def load_w(w_ext_, h, tagc, engines):
                wt = [wp.tile([P, D], f16, tag=f"{tagc}{h}_{k}", name=f"{tagc}{h}_{k}")
                      for k in range(ND)]
                for k in range(ND):
                    engines[k].dma_start(
                        out=wt[k][:],
                        in_=w_ext_[h * D + k * P: h * D + (k + 1) * P, :])
                return wt

            def dma_x(i):
                q = nc.gpsimd if i % 2 else nc.sync
                q.dma_start(out=xts[i][:], in_=x_ext[i * P:(i + 1) * P, :])

            # DMA issues cost ~600ns of engine time each: keep them off the
            # vector/scalar compute engines entirely
            for i in range(4):
                dma_x(i)
            ubt = cp.tile([P, 2 * ND], f32, tag="ubt")
            nc.sync.dma_start(out=ubt[:], in_=ub_ext[:, :])
            ident = cp.tile([P, P], f16, tag="ident")
            make_identity(nc, ident[:])
            at0 = load_w(a_ext, 0, "a", [nc.sync, nc.gpsimd, nc.sync, nc.gpsimd])
            for i in range(4, 7):
                dma_x(i)
            at1 = load_w(a_ext, 1, "a", [nc.sync, nc.gpsimd, nc.sync, nc.gpsimd])
            for i in range(7, 10):
                dma_x(i)
            nt0 = load_w(n_ext, 0, "n", [nc.sync, nc.gpsimd, nc.sync, nc.gpsimd])
            for i in range(10, 13):
                dma_x(i)
            nt1 = load_w(n_ext, 1, "n", [nc.sync, nc.gpsimd, nc.sync, nc.gpsimd])
            for i in range(13, NS):
                dma_x(i)

            eps = cp.tile([P, 1], f32, tag="eps")
            nc.vector.memset(eps[:], EPS)
            warm = cp.tile([P, 1], f32, tag="warm")
            nc.scalar.activation(out=warm[:], in_=eps[:],
                                 func=mybir.ActivationFunctionType.Sqrt,
                                 bias=eps[:], scale=1.0, alpha=0.0)
            zTb = xp.tile([P, ND, S], f16, tag="zt", name="zt")
            zT = [zTb[:, j, :] for j in range(ND)]
            qmT = [[qp.tile([P, S], f16, tag=f"qmt{h}_{j}", name=f"qmt{h}_{j}")
                    for j in range(ND)] for h in range(2)]
            HD = D // 2 + 1
            vm = [[qp.tile([P, 2, HD], bf16, tag=f"vm{h}_{t}", name=f"vm{h}_{t}")
                   for t in range(NS)] for h in range(2)]

            def emit_ln_group(g):
                # one batched Sqrt per 4 tiles keeps scalar's activation
                # table from thrashing between Sqrt and Exp
                mvs = []
                for q in range(4):
                    xt = xts[4 * g + q]
                    stats = lp.tile([P, 6], f32, tag="bs", name="bs")
                    nc.vector.bn_stats(out=stats[:], in_=xt[:])
                    mv = lp.tile([P, 2], f32, tag=f"mv{q}", name=f"mv{q}")
                    nc.vector.bn_aggr(out=mv[:], in_=stats[:])
                    mvs.append(mv)
                var4 = lp.tile([P, 4], f32, tag="var4", name="var4")
                for q in range(4):
                    nc.vector.tensor_copy(out=var4[:, q:q + 1], in_=mvs[q][:, 1:2])
                sd4 = lp.tile([P, 4], f32, tag="sd4", name="sd4")
                nc.scalar.activation(out=sd4[:], in_=var4[:],
                                     func=mybir.ActivationFunctionType.Sqrt,
                                     bias=eps[:], scale=1.0, alpha=0.0)
                nc.vector.reciprocal(out=sd4[:], in_=sd4[:])
                nb4 = lp.tile([P, 4], f32, tag="nb4", name="nb4")
                for q in range(4):
                    nc.vector.tensor_scalar(out=nb4[:, q:q + 1], in0=mvs[q][:, 0:1],
                                            scalar1=-1.0, scalar2=sd4[:, q:q + 1],
                                            op0=mybir.AluOpType.mult,
                                            op1=mybir.AluOpType.mult)
                for q in range(4):
                    i = 4 * g + q
                    xh = lp.tile([P, D], f16, tag="xh", name="xh")
                    nc.scalar.activation(out=xh[:], in_=xts[i][:],
                                         func=mybir.ActivationFunctionType.Identity,
                                         bias=nb4[:, q:q + 1], scale=sd4[:, q:q + 1])
                    tp = trlp.tile([P, D], f16, tag="tr", name="tp")
                    for j in range(ND):
                        nc.tensor.matmul(tp[:, j * P:(j + 1) * P],
                                         xh[:, j * P:(j + 1) * P], ident[:],
                                         is_transpose=True, skip_group_check=True)
                    nc.any.tensor_copy(out=zTb[:, :, i * P:(i + 1) * P], in_=tp[:])

            def emit_qm(h, at, g):
                # qmT[h][j][:, g*512:(g+1)*512] = A_h^T z^T + u
                for j in range(ND):
                    mm = mmp.tile([P, D], f32, tag="mm", name="mm")
                    for k in range(ND):
                        nc.tensor.matmul(mm[:],
                                         at[k][:, j * P:(j + 1) * P],
                                         zT[k][:, g * D:(g + 1) * D],
                                         start=(k == 0), stop=(k == ND - 1))
                    nc.any.tensor_scalar_add(out=qmT[h][j][:, g * D:(g + 1) * D],
                                             in0=mm[:],
                                             scalar1=ubt[:, h * ND + j:h * ND + j + 1])

            def emit_vm(h, nt, t):
                # vm[h][t] = [z N_h | ones] per 256-col half
                mm = mmp.tile([P, D], f32, tag="mm", name="mm")
                for k in range(ND):
                    nc.tensor.matmul(mm[:],
                                     zT[k][:, t * P:(t + 1) * P],
                                     nt[k][:, :],
                                     start=(k == 0), stop=(k == ND - 1))
                nc.gpsimd.memset(vm[h][t][:, :, HD - 1:HD], 1.0)
                nc.vector.tensor_copy(out=vm[h][t][:, :, 0:HD - 1], in_=mm[:])

            def emit_chunk_scores(h, c):
                # scoresT blocks [t-tile tb, s-chunk c], exp straight to SBUF.
                # diag block m only survives for s-cols >= m*128 (PV reads
                # slice r >= m; Z accumulates the same slice) so skip the rest
                nblk = 4 * c + 4
                pbs = []
                for tb in range(nblk):
                    m = tb - 4 * c
                    off = m * P if m >= 0 else 0
                    w = D - off
                    sc = scp.tile([P, D], f32, tag="sc", name="sc")
                    for k in range(ND):
                        nc.tensor.matmul(sc[:, 0:w],
                                         zT[k][:, tb * P:(tb + 1) * P],
                                         qmT[h][k][:, c * D + off:(c + 1) * D],
                                         start=(k == 0), stop=(k == ND - 1))
                    pb = pbp.tile([P, D], bf16, tag=f"pb{tb}", name=f"pb{tb}")
                    nc.scalar.activation(out=pb[:, off:D], in_=sc[:, 0:w],
                                         func=mybir.ActivationFunctionType.Exp,
                                         scale=1.0)
                    if m >= 0:
                        # zero the above-diagonal probs on the idle gpsimd;
                        # only the 128-wide boundary block is ever read dirty
                        nc.gpsimd.affine_select(
                            out=pb[:, off:off + P], in_=pb[:, off:off + P],
                            compare_op=mybir.AluOpType.is_ge,
                            fill=0.0, base=0,
                            pattern=[[1, P]],
                            channel_multiplier=-1,
                        )
                    pbs.append((pb, off))
                return pbs

            def emit_chunk_tails(h, c, pbs, final=False):
                rows = range(4)
                for r in rows:
                    i = 4 * c + r
                    pva = pvap.tile([P, HD], f32, tag="pva", name="pva")
                    pvb = pvbp.tile([P, HD], f32, tag="pvb", name="pvb")
                    for tb in range(i + 1):
                        nc.tensor.matmul(pva[:],
                                         pbs[tb][0][:, r * P:(r + 1) * P],
                                         vm[h][tb][:, 0, :],
                                         start=(tb == 0), stop=(tb == i))
                        nc.tensor.matmul(pvb[:],
                                         pbs[tb][0][:, r * P:(r + 1) * P],
                                         vm[h][tb][:, 1, :],
                                         start=(tb == 0), stop=(tb == i))
                    ot = up.tile([P, 2 * HD], bf16, tag="ot", name="ot")
                    nc.vector.tensor_copy(out=ot[:, 0:HD], in_=pva[:])
                    nc.vector.tensor_copy(out=ot[:, HD:2 * HD], in_=pvb[:])
                    nc.sync.dma_start(
                        out=out_ext[h * S + i * P:h * S + (i + 1) * P, :], in_=ot[:])

            def emit_ln_tile0(i):
                # group 0 runs per-tile (not batched) so the pipeline starts
                # as soon as x0 lands; no Exp has run yet so no table thrash
                stats = lp.tile([P, 6], f32, tag="bs", name="bs")
                nc.vector.bn_stats(out=stats[:], in_=xts[i][:])
                mv = lp.tile([P, 2], f32, tag=f"mv{i % 4}", name="mv")
                nc.vector.bn_aggr(out=mv[:], in_=stats[:])
                sd = lp.tile([P, 1], f32, tag="sd0", name="sd0")
                nc.scalar.activation(out=sd[:], in_=mv[:, 1:2],
                                     func=mybir.ActivationFunctionType.Sqrt,
                                     bias=eps[:], scale=1.0, alpha=0.0)
                nc.vector.reciprocal(out=sd[:], in_=sd[:])
                nb = lp.tile([P, 1], f32, tag="nb0", name="nb0")
                nc.vector.tensor_scalar(out=nb[:], in0=mv[:, 0:1],
                                        scalar1=-1.0, scalar2=sd[:],
                                        op0=mybir.AluOpType.mult,
                                        op1=mybir.AluOpType.mult)
                xh = lp.tile([P, D], f16, tag="xh", name="xh")
                nc.scalar.activation(out=xh[:], in_=xts[i][:],
                                     func=mybir.ActivationFunctionType.Identity,
                                     bias=nb[:], scale=sd[:])
                tp = trlp.tile([P, D], f16, tag="tr", name="tp")
                for j in range(ND):
                    nc.tensor.matmul(tp[:, j * P:(j + 1) * P],
                                     xh[:, j * P:(j + 1) * P], ident[:],
                                     is_transpose=True, skip_group_check=True)
                nc.any.tensor_copy(out=zTb[:, :, i * P:(i + 1) * P], in_=tp[:])

            # ---- phase A: LayerNorm interleaved with both heads' qm and
            #      head-0 vm (vm tile t only needs zT tile t) ----
            for i in range(4):
                emit_ln_tile0(i)
            for g in range(1, 4):
                emit_ln_group(g)
                emit_qm(0, at0, g - 1)
                emit_qm(1, at1, g - 1)
                if g == 2:
                    for t in range(0, 4):
                        emit_vm(0, nt0, t)
                    # head-0 chunk 0 scores ride under the remaining LN work
                    pend = (0, 0, emit_chunk_scores(0, 0))
                if g == 3:
                    for t in range(4, 8):
                        emit_vm(0, nt0, t)
            emit_qm(0, at0, 3)
            emit_qm(1, at1, 3)
            for t in range(8, 12):
                emit_vm(0, nt0, t)

            # ---- PE filler queue ----
            filler = [(0, nt0, t) for t in range(12, NS)] + \
                     [(1, nt1, t) for t in range(NS)]
            fpos = 0

            # ---- attention chunks: both heads ascending; head-1's early
            #      chunks overlap head-0's big final tails, and the kernel
            #      drains inside chunk (1,3)'s large PV instead of idling ----
            order = [(0, c) for c in range(1, NC_)] + [(1, c) for c in range(NC_)]
            for h, c in order:
                pbs = emit_chunk_scores(h, c)
                # all vm0 before tails(0,3), vm1[0..11] before tails(1,2);
                # the last 4 vm1 held back as late PE filler for the drain
                npop = 3 if (len(filler) - fpos > 4 or (h, c) == (1, NC_ - 1)) \
                    else 0
                if (h, c) == (1, NC_ - 1):
                    npop = len(filler) - fpos
                for _ in range(npop):
                    if fpos < len(filler):
                        emit_vm(*filler[fpos])
                        fpos += 1
                if pend is not None:
                    emit_chunk_tails(*pend)
                pend = (h, c, pbs)
            emit_chunk_tails(*pend, final=True)
    return nc


_NC = None


def _get_nc():
    global _NC
    if _NC is None:
        _NC = _build()
    return _NC


def _run(inputs, trace=False):
    x = np.asarray(inputs["x"], dtype=np.float32)          # [4, 2048, 512]
    gamma = np.asarray(inputs["gamma"], dtype=np.float32).reshape(D)
    beta = np.asarray(inputs["beta"], dtype=np.float32).reshape(D)
    Wq = np.asarray(inputs["Wq"], dtype=np.float32)        # [4, 512, 1024]
    Wk = np.asarray(inputs["Wk"], dtype=np.float32)
    Wv = np.asarray(inputs["Wv"], dtype=np.float32)
    Wout = np.asarray(inputs["Wout"], dtype=np.float32)    # [4096, 512]

    # Rank-D refactor: per head fold the QK^T and V-proj/out-proj pairs into
    # D x D matrices (U = 2D > D, so this more than halves the matmul work):
    #   scores = (z A + u) z^T      A = (g*Wq)(g*Wk)^T,  u = (b Wq)(g*Wk)^T
    #   head @ Wout = probs (z N) + (b Wv) Wout   N = (g*Wv) Wout
    # LN beta terms on the query side cancel in softmax; (b Wv) Wout is a
    # constant vector added host-side. Device returns unnormalized probs@vm
    # and the softmax row-sums; normalization happens here.
    H = 4
    A = np.empty((H, D, D), np.float32)
    Nm = np.empty((H, D, D), np.float32)
    ubias = np.empty((H, D), np.float32)
    cvec = np.zeros(D, np.float32)
    for h in range(H):
        Wkg = Wk[h] * gamma[:, None]
        A[h] = (Wq[h] * gamma[:, None]) @ Wkg.T
        ubias[h] = (beta @ Wq[h]) @ Wkg.T
        Nm[h] = (Wv[h] * gamma[:, None]) @ Wout[h * U:(h + 1) * U]
        cvec += (beta @ Wv[h]) @ Wout[h * U:(h + 1) * U]

    in_maps = []
    for c in range(8):
        b, hp = c // 2, c % 2
        ub = ubias[2 * hp:2 * hp + 2].reshape(2, ND, P).transpose(2, 0, 1).reshape(P, 2 * ND)
        in_maps.append({
            "x": np.ascontiguousarray(x[b]),
            "ub": np.ascontiguousarray(ub),
            "a": np.ascontiguousarray(A[2 * hp:2 * hp + 2].reshape(2 * D, D)).astype(np.float16),
            "n": np.ascontiguousarray(Nm[2 * hp:2 * hp + 2].reshape(2 * D, D)).astype(np.float16),
        })
    res = run_bass_kernel_spmd(_get_nc(), in_maps, list(range(8)), trace=trace)
    out = np.empty((4, S, D), np.float32)
    for b in range(4):
        acc = np.full((S, D), 0.0, np.float32)
        for hp in range(2):
            r = res.results[2 * b + hp]
            raw = np.asarray(r["out"]).astype(np.float32).reshape(2 * S, 2, D // 2 + 1)
            for h in range(2):
                blk = raw[h * S:(h + 1) * S]     # [S, 2, 257]
                zs = blk[:, 0, D // 2]           # softmax row sums
                acc += blk[:, :, 0:D // 2].reshape(S, D) / zs[:, None]
        out[b] = acc + cvec[None, :]
    return out, res


def kernel(**inputs):
    out, _ = _run(inputs, trace=False)
    return out


# revision 26
# speedup vs baseline: 1.1777x; 1.1777x over previous
import numpy as np
import concourse.bass as bass
import concourse.tile as tile
from concourse import mybir
from concourse.bass_utils import run_bass_kernel_spmd
from concourse.masks import make_identity

P = 128
S = 2048
D = 512
U = 1024
NS = S // P      # 16 s-tiles
ND = D // P      # 4 d-blocks
NC_ = S // D     # 4 s-chunks
NEG = -60000.0
EPS = 1e-6


def _patched_drain_and_barrier(self, tick_clock, wait_clock):
    nc = self.nc
    probe = nc.sync.nop(nofuse=True, hint="drain_waits_probe")
    wait_clock.add_sem_waits(probe.ins, tile.ScopedClock({None: tick_clock.global_clock}))
    si = probe.ins.sync_info
    waits = list(si.on_wait) if si is not None else []
    assert self.sems is not None
    handles = {h.name: h for h in self.sems.allocated().values()}
    if len(waits) > 1:
        import bass_rust
        probe.ins.sync_info = bass_rust.SyncInfo(on_wait=waits[:1], on_update=[])
        for w in waits[1:]:
            h = handles.get(w.ant_name)
            assert h is not None, (w.ant_name, list(handles))
            nc.sync.wait_ge(h, w.wait_value)
    nc.sync.drain()
    nc.all_engine_barrier()
    popped = nc._tile_sem_poison_stack.pop()
    assert popped is self._sem_poison
    nc.clear_and_free_semaphores(list(self.sems.allocated().values()))
    nc.all_engine_barrier()


tile.TileContext._drain_and_barrier = _patched_drain_and_barrier

# The walrus backend in this toolchain rejects instructions carrying more
# than one semaphore wait ("Too many sync wait commands"). Split excess
# waits onto single-wait NoOp carriers on the same engine, which execute
# in order ahead of the real instruction.
_MAXW = 1
_orig_lower_ordered = tile.TileContext._lower_ordered_insts


def _patched_lower_ordered(self, ordered):
    nc = self.nc
    for insts in ordered.values():
        out = []
        for inst in insts:
            si = getattr(inst, "sync_info", None)
            eng = getattr(inst, "engine", None)
            if (si is not None and si.on_wait and len(si.on_wait) > _MAXW
                    and eng is not None
                    and not type(inst).__name__.startswith("BassTile")):
                waits = list(si.on_wait)
                for w in waits[:-_MAXW]:
                    out.append(mybir.InstNoOp(
                        name=nc.get_next_instruction_name(),
                        engine=eng,
                        ins=[],
                        outs=[],
                        bass_nofuse=True,
                        sync_info=mybir.SyncInfo(on_wait=[w], on_update=[]),
                    ))
                inst.sync_info = mybir.SyncInfo(
                    on_wait=waits[-_MAXW:], on_update=list(si.on_update))
            out.append(inst)
        insts[:] = out
    return _orig_lower_ordered(self, ordered)


tile.TileContext._lower_ordered_insts = _patched_lower_ordered

f32 = mybir.dt.float32
f16 = mybir.dt.float16
bf16 = mybir.dt.bfloat16


def _build():
    nc = bass.Bass()
    # Per-core inputs (1 batch element, 2 heads):
    #   x   [S, D]    activations
    #   ub  [P, 2*ND] per-head score key-side bias (beta @ Wq @ (g*Wk)^T),
    #                 column h*ND+j holds entries d = j*128 + p
    #   a   [2D, D]   A_h = (g*Wq_h)(g*Wk_h)^T stacked over the 2 heads, f16
    #   n   [2D, D]   N_h = (g*Wv_h) Wout_h stacked, f16
    # scoresT = z (z A + u)^T computed transposed so exp writes probsT
    # directly; outputs are UNNORMALIZED probs@vm per head plus the softmax
    # row-sums z-vector; the host divides and sums heads.
    x_ext = nc.declare_dram_parameter("x", [S, D], f32, isOutput=False)
    ub_ext = nc.declare_dram_parameter("ub", [P, 2 * ND], f32, isOutput=False)
    a_ext = nc.declare_dram_parameter("a", [2 * D, D], f16, isOutput=False)
    n_ext = nc.declare_dram_parameter("n", [2 * D, D], f16, isOutput=False)
    # each row: [d0..d255, rowsum, d256..d511, rowsum] (PV folds the softmax
    # denominator in via a ones-column in vm)
    out_ext = nc.declare_dram_parameter("out", [2 * S, 2 * (D // 2 + 1)], bf16, isOutput=True)

    with tile.TileContext(nc) as tc:
        with tc.tile_pool(name="const", bufs=1) as cp, \
             tc.tile_pool(name="znt", bufs=1) as xp, \
             tc.tile_pool(name="wp", bufs=1) as wp, \
             tc.tile_pool(name="qkv", bufs=1) as qp, \
             tc.tile_pool(name="pb", bufs=2) as pbp, \
             tc.tile_pool(name="ln", bufs=2) as lp, \
             tc.tile_pool(name="xd", bufs=16) as xdp, \
             tc.tile_pool(name="outp", bufs=3) as up, \
             tc.tile_pool(name="mm", bufs=2, space="PSUM") as mmp, \
             tc.tile_pool(name="sc", bufs=3, space="PSUM") as scp, \
             tc.tile_pool(name="pva", bufs=1, space="PSUM") as pvap, \
             tc.tile_pool(name="pvb", bufs=1, space="PSUM") as pvbp, \
             tc.tile_pool(name="trl", bufs=1, space="PSUM") as trlp:

            dmaq = [nc.sync, nc.scalar, nc.gpsimd]

            # ---- stage all DMAs up front: x tiles first, weights interleaved ----
            xts = [xdp.tile([P, D], f32, tag="x", name=f"xt{i}") for i in range(NS)]

            def load_w(w_ext_, h, tagc, engines):
                wt = [wp.tile([P, D], f16, tag=f"{tagc}{h}_{k}", name=f"{tagc}{h}_{k}")
                      for k in range(ND)]
                for k in range(ND):
                    engines[k].dma_start(
                        out=wt[k][:],
                        in_=w_ext_[h * D + k * P: h * D + (k + 1) * P, :])
                return wt

            def dma_x(i):
                q = nc.gpsimd if i % 2 else nc.sync
                q.dma_start(out=xts[i][:], in_=x_ext[i * P:(i + 1) * P, :])

            # DMA issues cost ~600ns of engine time each: keep them off the
            # vector/scalar compute engines entirely
            for i in range(4):
                dma_x(i)
            ubt = cp.tile([P, 2 * ND], f32, tag="ubt")
            nc.sync.dma_start(out=ubt[:], in_=ub_ext[:, :])
            ident = cp.tile([P, P], f16, tag="ident")
            make_identity(nc, ident[:])
            at0 = load_w(a_ext, 0, "a", [nc.sync, nc.gpsimd, nc.sync, nc.gpsimd])
            for i in range(4, 7):
                dma_x(i)
            at1 = load_w(a_ext, 1, "a", [nc.sync, nc.gpsimd, nc.sync, nc.gpsimd])
            for i in range(7, 10):
                dma_x(i)
            nt0 = load_w(n_ext, 0, "n", [nc.sync, nc.gpsimd, nc.sync, nc.gpsimd])
            for i in range(10, 13):
                dma_x(i)
            nt1 = load_w(n_ext, 1, "n", [nc.sync, nc.gpsimd, nc.sync, nc.gpsimd])
            for i in range(13, NS):
                dma_x(i)

            eps = cp.tile([P, 1], f32, tag="eps")
            nc.vector.memset(eps[:], EPS)
            warm = cp.tile([P, 1], f32, tag="warm")
            nc.scalar.activation(out=warm[:], in_=eps[:],
                                 func=mybir.ActivationFunctionType.Sqrt,
                                 bias=eps[:], scale=1.0, alpha=0.0)
            zTb = xp.tile([P, ND, S], f16, tag="zt", name="zt")
            zT = [zTb[:, j, :] for j in range(ND)]
            qmT = [[qp.tile([P, S], f16, tag=f"qmt{h}_{j}", name=f"qmt{h}_{j}")
                    for j in range(ND)] for h in range(2)]
            HD = D // 2 + 1
            vm = [[qp.tile([P, 2, HD], bf16, tag=f"vm{h}_{t}", name=f"vm{h}_{t}")
                   for t in range(NS)] for h in range(2)]

            def emit_ln_group(g):
                # one batched Sqrt per 4 tiles keeps scalar's activation
                # table from thrashing between Sqrt and Exp
                mvs = []
                for q in range(4):
                    xt = xts[4 * g + q]
                    stats = lp.tile([P, 6], f32, tag="bs", name="bs")
                    nc.vector.bn_stats(out=stats[:], in_=xt[:])
                    mv = lp.tile([P, 2], f32, tag=f"mv{q}", name=f"mv{q}")
                    nc.vector.bn_aggr(out=mv[:], in_=stats[:])
                    mvs.append(mv)
                var4 = lp.tile([P, 4], f32, tag="var4", name="var4")
                for q in range(4):
                    nc.vector.tensor_copy(out=var4[:, q:q + 1], in_=mvs[q][:, 1:2])
                sd4 = lp.tile([P, 4], f32, tag="sd4", name="sd4")
                nc.scalar.activation(out=sd4[:], in_=var4[:],
                                     func=mybir.ActivationFunctionType.Sqrt,
                                     bias=eps[:], scale=1.0, alpha=0.0)
                nc.vector.reciprocal(out=sd4[:], in_=sd4[:])
                nb4 = lp.tile([P, 4], f32, tag="nb4", name="nb4")
                for q in range(4):
                    nc.vector.tensor_scalar(out=nb4[:, q:q + 1], in0=mvs[q][:, 0:1],
                                            scalar1=-1.0, scalar2=sd4[:, q:q + 1],
                                            op0=mybir.AluOpType.mult,
                                            op1=mybir.AluOpType.mult)
                for q in range(4):
                    i = 4 * g + q
                    xh = lp.tile([P, D], f16, tag="xh", name="xh")
                    nc.scalar.activation(out=xh[:], in_=xts[i][:],
                                         func=mybir.ActivationFunctionType.Identity,
                                         bias=nb4[:, q:q + 1], scale=sd4[:, q:q + 1])
                    tp = trlp.tile([P, D], f16, tag="tr", name="tp")
                    for j in range(ND):
                        nc.tensor.matmul(tp[:, j * P:(j + 1) * P],
                                         xh[:, j * P:(j + 1) * P], ident[:],
                                         is_transpose=True, skip_group_check=True)
                    nc.any.tensor_copy(out=zTb[:, :, i * P:(i + 1) * P], in_=tp[:])

            def emit_qm(h, at, g):
                # qmT[h][j][:, g*512:(g+1)*512] = A_h^T z^T + u
                for j in range(ND):
                    mm = mmp.tile([P, D], f32, tag="mm", name="mm")
                    for k in range(ND):
                        nc.tensor.matmul(mm[:],
                                         at[k][:, j * P:(j + 1) * P],
                                         zT[k][:, g * D:(g + 1) * D],
                                         start=(k == 0), stop=(k == ND - 1))
                    nc.any.tensor_scalar_add(out=qmT[h][j][:, g * D:(g + 1) * D],
                                             in0=mm[:],
                                             scalar1=ubt[:, h * ND + j:h * ND + j + 1])

            def emit_vm(h, nt, t):
                # vm[h][t] = [z N_h | ones] per 256-col half
                mm = mmp.tile([P, D], f32, tag="mm", name="mm")
                for k in range(ND):
                    nc.tensor.matmul(mm[:],
                                     zT[k][:, t * P:(t + 1) * P],
                                     nt[k][:, :],
                                     start=(k == 0), stop=(k == ND - 1))
                nc.gpsimd.memset(vm[h][t][:, :, HD - 1:HD], 1.0)
                nc.vector.tensor_copy(out=vm[h][t][:, :, 0:HD - 1], in_=mm[:])

            def emit_chunk_scores(h, c):
                # scoresT blocks [t-tile tb, s-chunk c], exp straight to SBUF.
                # diag block m only survives for s-cols >= m*128 (PV reads
                # slice r >= m; Z accumulates the same slice) so skip the rest
                nblk = 4 * c + 4
                pbs = []
                for tb in range(nblk):
                    m = tb - 4 * c
                    off = m * P if m >= 0 else 0
                    w = D - off
                    sc = scp.tile([P, D], f32, tag="sc", name="sc")
                    for k in range(ND):
                        nc.tensor.matmul(sc[:, 0:w],
                                         zT[k][:, tb * P:(tb + 1) * P],
                                         qmT[h][k][:, c * D + off:(c + 1) * D],
                                         start=(k == 0), stop=(k == ND - 1))
                    pb = pbp.tile([P, D], bf16, tag=f"pb{tb}", name=f"pb{tb}")
                    nc.scalar.activation(out=pb[:, off:D], in_=sc[:, 0:w],
                                         func=mybir.ActivationFunctionType.Exp,
                                         scale=1.0)
                    if m >= 0:
                        # zero the above-diagonal probs on the idle gpsimd;
                        # only the 128-wide boundary block is ever read dirty
                        nc.gpsimd.affine_select(
                            out=pb[:, off:off + P], in_=pb[:, off:off + P],
                            compare_op=mybir.AluOpType.is_ge,
                            fill=0.0, base=0,
                            pattern=[[1, P]],
                            channel_multiplier=-1,
                        )
                    pbs.append((pb, off))
                return pbs

            def emit_chunk_tails(h, c, pbs, final=False):
                rows = range(4)
                for r in rows:
                    i = 4 * c + r
                    pva = pvap.tile([P, HD], f32, tag="pva", name="pva")
                    pvb = pvbp.tile([P, HD], f32, tag="pvb", name="pvb")
                    for tb in range(i + 1):
                        nc.tensor.matmul(pva[:],
                                         pbs[tb][0][:, r * P:(r + 1) * P],
                                         vm[h][tb][:, 0, :],
                                         start=(tb == 0), stop=(tb == i))
                        nc.tensor.matmul(pvb[:],
                                         pbs[tb][0][:, r * P:(r + 1) * P],
                                         vm[h][tb][:, 1, :],
                                         start=(tb == 0), stop=(tb == i))
                    ot = up.tile([P, 2 * HD], bf16, tag="ot", name="ot")
                    nc.vector.tensor_copy(out=ot[:, 0:HD], in_=pva[:])
                    nc.vector.tensor_copy(out=ot[:, HD:2 * HD], in_=pvb[:])
                    nc.sync.dma_start(
                        out=out_ext[h * S + i * P:h * S + (i + 1) * P, :], in_=ot[:])

            def emit_ln_tile0(i):
                # group 0 runs per-tile (not batched) so the pipeline starts
                # as soon as x0 lands; no Exp has run yet so no table thrash
                stats = lp.tile([P, 6], f32, tag="bs", name="bs")
                nc.vector.bn_stats(out=stats[:], in_=xts[i][:])
                mv = lp.tile([P, 2], f32, tag=f"mv{i % 4}", name="mv")
                nc.vector.bn_aggr(out=mv[:], in_=stats[:])
                sd = lp.tile([P, 1], f32, tag="sd0", name="sd0")
                nc.scalar.activation(out=sd[:], in_=mv[:, 1:2],
                                     func=mybir.ActivationFunctionType.Sqrt,
                                     bias=eps[:], scale=1.0, alpha=0.0)
                nc.vector.reciprocal(out=sd[:], in_=sd[:])
                nb = lp.tile([P, 1], f32, tag="nb0", name="nb0")
                nc.vector.tensor_scalar(out=nb[:], in0=mv[:, 0:1],
                                        scalar1=-1.0, scalar2=sd[:],
                                        op0=mybir.AluOpType.mult,
                                        op1=mybir.AluOpType.mult)
                xh = lp.tile([P, D], f16, tag="xh", name="xh")
                nc.scalar.activation(out=xh[:], in_=xts[i][:],
                                     func=mybir.ActivationFunctionType.Identity,
                                     bias=nb[:], scale=sd[:])
                tp = trlp.tile([P, D], f16, tag="tr", name="tp")
                for j in range(ND):
                    nc.tensor.matmul(tp[:, j * P:(j + 1) * P],
                                     xh[:, j * P:(j + 1) * P], ident[:],
                                     is_transpose=True, skip_group_check=True)
                nc.any.tensor_copy(out=zTb[:, :, i * P:(i + 1) * P], in_=tp[:])

            # ---- phase A: LayerNorm interleaved with both heads' qm and
            #      head-0 vm (vm tile t only needs zT tile t) ----
            for i in range(4):
                emit_ln_tile0(i)
            for g in range(1, 4):
                emit_ln_group(g)
                emit_qm(0, at0, g - 1)
                emit_qm(1, at1, g - 1)
                if g >= 2:
                    for t in range(4 * (g - 2), 4 * (g - 1)):
                        emit_vm(0, nt0, t)
            emit_qm(0, at0, 3)
            emit_qm(1, at1, 3)
            for t in range(8, NS):
                emit_vm(0, nt0, t)

            # ---- PE filler queue: head-1 vm tiles ----
            filler = [(1, nt1, t) for t in range(NS)]
            fpos = 0

            # ---- attention chunks: both heads ascending; head-1's early
            #      chunks overlap head-0's big final tails, and the kernel
            #      drains inside chunk (1,3)'s large PV instead of idling ----
            order = [(0, c) for c in range(NC_)] + [(1, c) for c in range(NC_)]
            pend = None
            for h, c in order:
                pbs = emit_chunk_scores(h, c)
                # vm1[0..11] before tails(1,2); the last 4 held back as
                # late PE filler for the drain
                npop = 3 if h == 0 else (len(filler) - fpos if c == NC_ - 1 else 0)
                for _ in range(npop):
                    if fpos < len(filler):
                        emit_vm(*filler[fpos])
                        fpos += 1
                if pend is not None:
                    emit_chunk_tails(*pend)
                pend = (h, c, pbs)
            emit_chunk_tails(*pend, final=True)
    return nc


_NC = None


def _get_nc():
    global _NC
    if _NC is None:
        _NC = _build()
    return _NC


def _run(inputs, trace=False):
    x = np.asarray(inputs["x"], dtype=np.float32)          # [4, 2048, 512]
    gamma = np.asarray(inputs["gamma"], dtype=np.float32).reshape(D)
    beta = np.asarray(inputs["beta"], dtype=np.float32).reshape(D)
    Wq = np.asarray(inputs["Wq"], dtype=np.float32)        # [4, 512, 1024]
    Wk = np.asarray(inputs["Wk"], dtype=np.float32)
    Wv = np.asarray(inputs["Wv"], dtype=np.float32)
    Wout = np.asarray(inputs["Wout"], dtype=np.float32)    # [4096, 512]

    # Rank-D refactor: per head fold the QK^T and V-proj/out-proj pairs into
    # D x D matrices (U = 2D > D, so this more than halves the matmul work):
    #   scores = (z A + u) z^T      A = (g*Wq)(g*Wk)^T,  u = (b Wq)(g*Wk)^T
    #   head @ Wout = probs (z N) + (b Wv) Wout   N = (g*Wv) Wout
    # LN beta terms on the query side cancel in softmax; (b Wv) Wout is a
    # constant vector added host-side. Device returns unnormalized probs@vm
    # and the softmax row-sums; normalization happens here.
    H = 4
    A = np.empty((H, D, D), np.float32)
    Nm = np.empty((H, D, D), np.float32)
    ubias = np.empty((H, D), np.float32)
    cvec = np.zeros(D, np.float32)
    for h in range(H):
        Wkg = Wk[h] * gamma[:, None]
        A[h] = (Wq[h] * gamma[:, None]) @ Wkg.T
        ubias[h] = (beta @ Wq[h]) @ Wkg.T
        Nm[h] = (Wv[h] * gamma[:, None]) @ Wout[h * U:(h + 1) * U]
        cvec += (beta @ Wv[h]) @ Wout[h * U:(h + 1) * U]

    in_maps = []
    for c in range(8):
        b, hp = c // 2, c % 2
        ub = ubias[2 * hp:2 * hp + 2].reshape(2, ND, P).transpose(2, 0, 1).reshape(P, 2 * ND)
        in_maps.append({
            "x": np.ascontiguousarray(x[b]),
            "ub": np.ascontiguousarray(ub),
            "a": np.ascontiguousarray(A[2 * hp:2 * hp + 2].reshape(2 * D, D)).astype(np.float16),
            "n": np.ascontiguousarray(Nm[2 * hp:2 * hp + 2].reshape(2 * D, D)).astype(np.float16),
        })
    res = run_bass_kernel_spmd(_get_nc(), in_maps, list(range(8)), trace=trace)
    out = np.empty((4, S, D), np.float32)
    for b in range(4):
        acc = np.full((S, D), 0.0, np.float32)
        for hp in range(2):
            r = res.results[2 * b + hp]
            raw = np.asarray(r["out"]).astype(np.float32).reshape(2 * S, 2, D // 2 + 1)
            for h in range(2):
                blk = raw[h * S:(h + 1) * S]     # [S, 2, 257]
                zs = blk[:, 0, D // 2]           # softmax row sums
                acc += blk[:, :, 0:D // 2].reshape(S, D) / zs[:, None]
        out[b] = acc + cvec[None, :]
    return out, res


def kernel(**inputs):
    out, _ = _run(inputs, trace=False)
    return out
